# revision 27
# baseline (speedup 1.0000x reference)
"""LlamaSkipMLP Trainium2 kernel.

Strategy: data-parallel over the token dim across 8 NeuronCores (no
collectives).  Each core computes out_c = silu(x_c@Wg'.T) * (x_c@Wu'.T) @ Wd'.T
for its 1024-token slice, where Wg'/Wu'/Wd' are the active-neuron
gather of the weights (done host-side; for active_idx = arange(k) it
is a plain slice).

Device kernel (per core, Tile framework):
  phase 1: g/u GEMMs contract hidden dim H (on PE partitions), fused
           SiLU*up on ACT+DVE, h stored [k_part, t_free] in fp16.
           g and u matmuls interleave per h0 so the x^T stream is
           consumed at half rate during k0=0 (matches HBM bandwidth).
  phase 2: down GEMM contracts the active-neuron dim k; h tiles serve
           as the stationary operand, W_down^T tiles as the moving
           operand, so the output lands as [t_part, h_free] and stores
           contiguously in bf16.  Token tiles run in groups of four
           (tapering to 3+1 for the last hf) sharing this hf's
           SBUF-resident wd stream, so PSUM drains and output DMAs
           pipeline against the next group's matmuls and the kernel
           tail after the last matmul is ~2us.

A warmup burst of throwaway matmuls runs while the first DMAs are in
flight so the PE's HAM clock-gate is released before real data lands.

All matmuls run in fp16 (PSUM accumulates fp32).  Host pre-lays-out
weights/activations so every DMA is contiguous and no on-device
transposes are needed.
"""

import numpy as np

# Problem shapes (hardcoded per spec).
T, H, K = 8192, 4096, 3302
NCORES = 8
KP = 3328                 # K padded to a multiple of 128
NK0 = KP // 128           # 26 k-tiles
NH0 = H // 128            # 32 h-tiles (contraction, phase 1)
TC = T // NCORES          # 1024 tokens per core

_CACHE = {}


def build_nc(kp=KP, h=H, tct=TC, enable_asserts=False):
    """Build + compile the per-core Bass program (SPMD: same on all cores)."""
    from contextlib import ExitStack

    import concourse.mybir as mybir
    import concourse.tile as tile
    from concourse import bacc

    fp16 = mybir.dt.float16
    fp32 = mybir.dt.float32
    Sigmoid = mybir.ActivationFunctionType.Sigmoid
    Copy = mybir.ActivationFunctionType.Copy

    nk0 = kp // 128
    nh0 = h // 128
    ntf = tct // 512          # moving t-tiles, phase 1
    nt1 = tct // 128          # stationary t-tiles, phase 2
    nhf = h // 512            # hf chunks, phase 2
    kh = nk0 // 2             # k0 half-stream length, phase 2 (13)

    nc = bacc.Bacc(
        "TRN2", target_bir_lowering=False, debug=False,
        enable_asserts=enable_asserts,
    )
    xt = nc.dram_tensor("xt", [128, nh0 * tct], fp16, kind="ExternalInput").ap()
    wg = nc.dram_tensor("wg", [nk0, 128, nh0 * 128], fp16, kind="ExternalInput").ap()
    wu = nc.dram_tensor("wu", [nk0, 128, nh0 * 128], fp16, kind="ExternalInput").ap()
    # phase-2 weights, pre-grouped per (hf, k-half): [nhf, 2, 128, 13*512]
    wd = nc.dram_tensor("wd", [nhf, 2, 128, kh * 512], fp16,
                        kind="ExternalInput").ap()
    bf16 = mybir.dt.bfloat16
    out = nc.dram_tensor("out", [tct, h], bf16, kind="ExternalOutput").ap()

    with tile.TileContext(nc) as tc, ExitStack() as ctx:
        xt_pool = ctx.enter_context(tc.tile_pool(name="xtp", bufs=1))
        w_pool = ctx.enter_context(tc.tile_pool(name="wp", bufs=2))
        wd_pool = ctx.enter_context(tc.tile_pool(name="wdp", bufs=2))
        h_pool = ctx.enter_context(tc.tile_pool(name="hp", bufs=1))
        tmp_pool = ctx.enter_context(tc.tile_pool(name="tmpp", bufs=2))
        out_pool = ctx.enter_context(tc.tile_pool(name="outp", bufs=8))
        warm_pool = ctx.enter_context(tc.tile_pool(name="warmp", bufs=1))

        xt_sb = xt_pool.tile([128, nh0 * tct], fp16, name="xt_sb")
        # h is split in two tiles so phase 2's first matmuls only wait
        # on the first half's writes (tile-granular cross-engine deps).
        h_sba = h_pool.tile([128, kh * tct], fp16, name="h_sba")
        h_sbb = h_pool.tile([128, (nk0 - kh) * tct], fp16, name="h_sbb")

        def h_slice(k0, lo, hi):
            t, kk = (h_sba, k0) if k0 < kh else (h_sbb, k0 - kh)
            return t[:, kk * tct + lo:kk * tct + hi]

        # ---- warmup: keep the PE busy while the first DMAs fly so the
        # HAM clock-gate releases before real data lands.
        # FD=128 warmup matmuls: the tiny memset (256B/partition) clears
        # fast, and the 107ns-cold / 53ns-warm granularity lets the warmup
        # burst end right when the first real data lands.
        wtile = warm_pool.tile([128, 128], fp16, name="wtile")
        nc.gpsimd.memset(wtile[:, :], 0.0)
        nwarm = 88
        with tc.tile_pool(name="psw", space="PSUM", bufs=1) as psw:
            pw = psw.tile([128, 128], fp32, name="pwarm")
            for w in range(nwarm):
                nc.tensor.matmul(pw[:, :], wtile[:, :], wtile[:, :],
                                 start=(w == 0), stop=(w == nwarm - 1))

        # ---- startup DMAs: quarter-slabs of wg[0]/wu[0] interleaved with
        # x^T chunks, ordered so the k0=0 sweep never starves.
        wg_t0 = w_pool.tile([128, nh0 * 128], fp16, name="wg_t", tag="wg")
        wu_t0 = w_pool.tile([128, nh0 * 128], fp16, name="wu_t", tag="wu")
        nchunk = 16
        csz = nh0 * tct // nchunk            # 2 h0-columns per chunk
        qsz = nh0 * 128 // 4                 # quarter-slab of a weight
        # Pieces issued in consumption (need-time) order: the k0=0 sweep's
        # demand (x + wg0 + wu0 = 10.5MB) exactly matches the DMA ramp, so
        # ordering by need keeps the PE from stalling.
        def w0_piece(which, lo, hi, eng=None):
            t, src_ = (wg_t0, wg) if which == "g" else (wu_t0, wu)
            (eng or nc.sync).dma_start(t[:, lo:hi], src_[0][:, lo:hi])

        def x_chunk(c):
            nc.sync.dma_start(xt_sb[:, c * csz:(c + 1) * csz],
                              xt[:, c * csz:(c + 1) * csz])

        # First wg/wu quarters issue on the scalar queue in parallel with
        # sync's x chunks, so the first pieces reach the DMA engines sooner.
        w0_piece("g", 0, qsz, eng=nc.scalar)
        w0_piece("u", 0, qsz, eng=nc.scalar)
        for c in (0, 1, 2, 3):
            x_chunk(c)
        w0_piece("g", qsz, 2 * qsz)
        x_chunk(4)
        w0_piece("u", qsz, 2 * qsz)
        for c in (5, 6, 7):
            x_chunk(c)
        w0_piece("g", 2 * qsz, 4 * qsz)
        x_chunk(8)
        w0_piece("u", 2 * qsz, 4 * qsz)
        for c in range(9, nchunk):
            x_chunk(c)

        # ---- phase 1: g = x@Wg^T, u = x@Wu^T, h = silu(g)*u ----
        with tc.tile_pool(name="ps1", space="PSUM", bufs=2) as ps1:
            for k0 in range(nk0):
                if k0 == 0:
                    wg_t, wu_t = wg_t0, wu_t0
                else:
                    wg_t = w_pool.tile([128, nh0 * 128], fp16, name="wg_t", tag="wg")
                    nc.sync.dma_start(wg_t[:, :], wg[k0])
                    wu_t = w_pool.tile([128, nh0 * 128], fp16, name="wu_t", tag="wu")
                    nc.sync.dma_start(wu_t[:, :], wu[k0])
                pg = [ps1.tile([128, 512], fp32, name=f"pg{i}", tag=f"pg{i}")
                      for i in range(ntf)]
                pu = [ps1.tile([128, 512], fp32, name=f"pu{i}", tag=f"pu{i}")
                      for i in range(ntf)]

                def ew(i):
                    sg = tmp_pool.tile([128, 512], fp32, name="sg", tag="sg")
                    nc.scalar.activation(sg[:, :], pg[i][:, :], Sigmoid)
                    sl = tmp_pool.tile([128, 512], fp32, name="sl", tag="sl")
                    nc.vector.tensor_mul(sl[:, :], sg[:, :], pg[i][:, :])
                    nc.vector.tensor_mul(
                        h_slice(k0, i * 512, (i + 1) * 512),
                        sl[:, :], pu[i][:, :])

                if k0 < nk0 - 1:
                    # h0-outer, g/u interleaved: x chunk h0 is consumed by
                    # 4 matmuls before moving on (DMA-paced at k0=0).
                    for h0 in range(nh0):
                        for i in range(ntf):
                            nc.tensor.matmul(
                                pg[i][:, :], wg_t[:, h0 * 128:(h0 + 1) * 128],
                                xt_sb[:, h0 * tct + i * 512:h0 * tct + (i + 1) * 512],
                                start=(h0 == 0), stop=(h0 == nh0 - 1),
                            )
                        for i in range(ntf):
                            nc.tensor.matmul(
                                pu[i][:, :], wu_t[:, h0 * 128:(h0 + 1) * 128],
                                xt_sb[:, h0 * tct + i * 512:h0 * tct + (i + 1) * 512],
                                start=(h0 == 0), stop=(h0 == nh0 - 1),
                            )
                    for i in range(ntf):
                        ew(i)
                else:
                    # last k0: i-outer so the first PSUM banks drain while
                    # the second half still streams (shrinks the phase gap).
                    for i in range(ntf):
                        for h0 in range(nh0):
                            nc.tensor.matmul(
                                pg[i][:, :], wg_t[:, h0 * 128:(h0 + 1) * 128],
                                xt_sb[:, h0 * tct + i * 512:h0 * tct + (i + 1) * 512],
                                start=(h0 == 0), stop=(h0 == nh0 - 1),
                            )
                        for h0 in range(nh0):
                            nc.tensor.matmul(
                                pu[i][:, :], wu_t[:, h0 * 128:(h0 + 1) * 128],
                                xt_sb[:, h0 * tct + i * 512:h0 * tct + (i + 1) * 512],
                                start=(h0 == 0), stop=(h0 == nh0 - 1),
                            )
                        ew(i)

        # ---- phase 2: out = h @ Wd^T (contract k) ----
        # Token tiles run in groups (4, last hf in 2s) so drains + output
        # DMAs pipeline against the next group's matmuls.
        with tc.tile_pool(name="ps2", space="PSUM", bufs=2) as ps2:
            def p2_group(hf, t1s, tags, wd_halves):
                """One token-tile group: wd_halves are the two SBUF tiles
                holding this hf's full k-stream (loaded by the hf's first
                group; later groups reuse the same buffers — no re-DMA)."""
                po = [ps2.tile([128, 512], fp32, name=f"po{t}", tag=tags[j])
                      for j, t in enumerate(t1s)]
                for half in range(2):
                    wd_t = wd_halves[half]
                    for kk in range(kh):
                        k0 = half * kh + kk
                        for j, t1 in enumerate(t1s):
                            nc.tensor.matmul(
                                po[j][:, :],
                                h_slice(k0, t1 * 128, (t1 + 1) * 128),
                                wd_t[:, kk * 512:(kk + 1) * 512],
                                start=(k0 == 0), stop=(k0 == nk0 - 1),
                            )
                if hf == nhf - 1 and len(t1s) == 1:
                    # very last tile: drain halves on DVE+ACT in parallel and
                    # store via two queues so the tail past the last matmul
                    # is as short as possible.
                    t1 = t1s[0]
                    ota = out_pool.tile([128, 256], bf16, name="ota", tag="ota")
                    otb = out_pool.tile([128, 256], bf16, name="otb", tag="otb")
                    nc.vector.tensor_copy(ota[:, :], po[0][:, 0:256])
                    nc.scalar.activation(otb[:, :], po[0][:, 256:512], Copy)
                    r = slice(t1 * 128, (t1 + 1) * 128)
                    nc.sync.dma_start(out[r, hf * 512:hf * 512 + 256], ota[:, :])
                    nc.scalar.dma_start(out[r, hf * 512 + 256:(hf + 1) * 512],
                                        otb[:, :])
                    return
                for j, t1 in enumerate(t1s):
                    ot = out_pool.tile([128, 512], bf16, name="ot", tag="ot")
                    if j % 2 == 0:
                        nc.vector.tensor_copy(ot[:, :], po[j][:, :])
                    else:
                        nc.scalar.activation(ot[:, :], po[j][:, :], Copy)
                    nc.sync.dma_start(
                        out[t1 * 128:(t1 + 1) * 128, hf * 512:(hf + 1) * 512],
                        ot[:, :])

            for hf in range(nhf):
                wd_halves = []
                for half in range(2):
                    wd_t = wd_pool.tile([128, kh * 512], fp16, name="wd_t",
                                        tag="wd")
                    nc.sync.dma_start(wd_t[:, :], wd[hf, half])
                    wd_halves.append(wd_t)
                p2_group(hf, [0, 1, 2, 3], ["pa", "pb", "pc", "pd"], wd_halves)
                if hf < nhf - 1:
                    p2_group(hf, [4, 5, 6, 7], ["pa", "pb", "pc", "pd"],
                             wd_halves)
                else:
                    # last hf: taper group sizes so the final drain + store
                    # trail only ~2us past the last matmul.
                    p2_group(hf, [4, 5, 6], ["pa", "pb", "pc"], wd_halves)
                    p2_group(hf, [7], ["pd"], wd_halves)

    nc.compile()
    return nc


def prep_weights(W_gate, W_up, W_down, active_idx, kp=KP, h=H):
    idx = np.asarray(active_idx)
    k = idx.shape[0]
    nk0 = kp // 128
    nh0 = h // 128
    nhf = h // 512
    kh = nk0 // 2

    def lay_gu(W):
        a = np.zeros((kp, h), np.float16)
        a[:k] = W[idx].astype(np.float16)
        # [k0, p, h0*128 + k_in] = a[k0*128+k_in, h0*128+p]
        return np.ascontiguousarray(
            a.reshape(nk0, 128, nh0, 128).transpose(0, 3, 2, 1)
        ).reshape(nk0, 128, nh0 * 128)

    wd_a = np.zeros((kp, h), np.float16)
    wd_a[:k] = W_down[:, idx].T.astype(np.float16)
    # [hf, half, p, kk*512+e] = wd_a[(half*13+kk)*128+p, hf*512+e]
    wd_prep = np.ascontiguousarray(
        wd_a.reshape(2, kh, 128, nhf, 512).transpose(3, 0, 2, 1, 4)
    ).reshape(nhf, 2, 128, kh * 512)
    return lay_gu(W_gate), lay_gu(W_up), wd_prep


def prep_x_core(xc, h=H, tct=TC):
    nh0 = h // 128
    xt_c = np.ascontiguousarray(
        xc.astype(np.float16).T.reshape(nh0, 128, tct).transpose(1, 0, 2))
    return xt_c.reshape(128, nh0 * tct)


def run(inputs, trace=False, **kw):
    from concourse.bass_utils import run_bass_kernel_spmd

    if "nc" not in _CACHE:
        _CACHE["nc"] = build_nc()
    nc = _CACHE["nc"]

    wg_prep, wu_prep, wd_prep = prep_weights(
        inputs["W_gate"], inputs["W_up"], inputs["W_down"], inputs["active_idx"])
    x = inputs["x"]
    in_maps = [
        {"xt": prep_x_core(x[c * TC:(c + 1) * TC]),
         "wg": wg_prep, "wu": wu_prep, "wd": wd_prep}
        for c in range(NCORES)
    ]
    res = run_bass_kernel_spmd(nc, in_maps, core_ids=list(range(NCORES)),
                               trace=trace, **kw)
    out = np.concatenate([res.results[c]["out"].astype(np.float32)
                          for c in range(NCORES)], axis=0)
    return out, res


def kernel(**inputs):
    out, _ = run(inputs, trace=False)
    return out
